# revision 5
# baseline (speedup 1.0000x reference)
"""Raw-bacc MaxPool3d kernel, v8: bf16 + rotating load split + 3-op DVE chain.

bf16 datapath (v7): max-pool commutes with monotone bf16 rounding, so
pooling bf16-rounded inputs yields exactly bf16(true_max) — rel err
<= 2^-8, far inside the 2e-2 gate — and halves HBM traffic, which is the
binding constraint (16 DMA engines/core x ~26.5 GB/s, all saturated).

v8 changes:
- Rotating load split: each parity load is issued as two dma_starts with
  a per-tile rotating plane boundary. The DGE assigns each engine a fixed
  slice of every dma_start, so without rotation engine j always reads the
  same address stripe of every tile; a contended stripe then turns one
  engine into a run-long straggler (observed: +16-19% busy on one engine,
  different engine each run). Rotation spreads any hot stripe over all 16
  engines.
- 3-op DVE chain per tile: D-pair max (a0 vs a1), then H-pair, then
  W-pair straight into the store tile. Same element count as the old
  5-op per-parity chain but one op per reduction level, cutting DVE busy
  ~89 -> ~76 us so DVE never gates the DMA stream.
- Final tile split in half by free rows (loads, DVE chain, and store), so
  the post-last-packet tail is one half-chain + half-store.
"""

import numpy as np
from ml_dtypes import bfloat16

import concourse.bass as bass
from concourse import bacc, mybir
from concourse import bass_utils

CPC = 8
D = H = W = 128
DT = mybir.dt.bfloat16
NSLOT = 4
NT = 16

_CACHE = {}


def _build_module():
    nc = bacc.Bacc("TRN2", target_bir_lowering=False, debug=False, num_devices=8)
    x = nc.dram_tensor("x", [CPC, D, H, W], DT, kind="ExternalInput").ap()
    y = nc.dram_tensor("y", [CPC, D // 2, H // 2, W // 2], DT, kind="ExternalOutput").ap()

    a0 = [nc.alloc_sbuf_tensor(f"a0_{i}", [128, 32, 128], DT).ap() for i in range(NSLOT)]
    a1 = [nc.alloc_sbuf_tensor(f"a1_{i}", [128, 32, 128], DT).ap() for i in range(NSLOT)]
    dm = nc.alloc_sbuf_tensor("dm", [128, 32, 128], DT).ap()
    hm = nc.alloc_sbuf_tensor("hm", [128, 16, 128], DT).ap()
    wm = [nc.alloc_sbuf_tensor(f"wm_{i}", [128, 16, 64], DT).ap() for i in range(2)]

    a0_sems = [nc.alloc_semaphore(f"a0_sem{i}") for i in range(NSLOT)]
    a1_sems = [nc.alloc_semaphore(f"a1_sem{i}") for i in range(NSLOT)]
    wm_sems = [nc.alloc_semaphore(f"wm_sem{i}") for i in range(2)]
    fh_sem = nc.alloc_semaphore("fh_sem")
    rel_sem = nc.alloc_semaphore("rel_sem")
    comp_sem = nc.alloc_semaphore("comp_sem")
    compl_sem = nc.alloc_semaphore("compl_sem")

    def tile_slices(t):
        c, half = divmod(t, 2)
        return c, half * 64

    # Plane-boundary rotation per tile: both sub-DMAs stay >= 4 planes
    # (128 KiB) so every engine participates and each dma_start's 16 sem
    # increments are all delivered.
    rot = [4 + (5 * t + 3) % 25 for t in range(NT)]

    # --- SP: loads -----------------------------------------------------
    for t in range(NT):
        c, base = tile_slices(t)
        k = t % NSLOT
        if t >= NSLOT:
            nc.sync.wait_ge(rel_sem, t - NSLOT + 1)
        if t < NT - 1:
            r = rot[t]
            for par, buf, sems in ((0, a0, a0_sems), (1, a1, a1_sems)):
                lo = base + par
                nc.sync.dma_start(
                    buf[k][4 * r : 128], x[c, lo + 2 * r : base + 64 : 2]
                ).then_inc(sems[k], 16)
                nc.sync.dma_start(
                    buf[k][0 : 4 * r], x[c, lo : lo + 2 * r : 2]
                ).then_inc(sems[k], 16)
        else:
            # final tile: halve every load by free rows (h%32 stripes) so
            # the DVE chain and store can run on the first half while the
            # second is in flight.
            for par, buf, sems in ((0, a0, a0_sems), (1, a1, a1_sems)):
                src = x[c, base + par : base + 64 : 2]
                sr = src.rearrange("d (hb r) w -> d hb (r w)", hb=4)
                nc.sync.dma_start(buf[k][:, 0:16, :], sr[:, :, 0:2048]).then_inc(
                    sems[k], 16
                )
                nc.sync.dma_start(buf[k][:, 16:32, :], sr[:, :, 2048:4096]).then_inc(
                    fh_sem, 16
                )

    # --- DVE: D-pair, H-pair, W-pair max -------------------------------
    def chain(dst, rows, n):
        # rows: dm/a row offset; n: dm/a row count (out rows n//4 in dst)
        dmv = dm[:, rows : rows + n, :]
        nc.vector.tensor_max(dmv, a0_v[:, rows : rows + n, :], a1_v[:, rows : rows + n, :])
        hv = hm[:, rows // 2 : rows // 2 + n // 2, :]
        nc.vector.tensor_max(hv, dmv[:, 0::2, :], dmv[:, 1::2, :])
        wp = hv.rearrange("p r (w2 two) -> p r w2 two", two=2)
        return nc.vector.tensor_max(dst, wp[:, :, :, 0], wp[:, :, :, 1])

    wm_uses = [0, 0]
    for t in range(NT):
        k = t % NSLOT
        m = t % 2
        uses = t // NSLOT + 1
        a0_v, a1_v = a0[k], a1[k]
        if t < NT - 1:
            nc.vector.wait_ge(a0_sems[k], 32 * uses)
            nc.vector.wait_ge(a1_sems[k], 32 * uses)
            nc.vector.tensor_max(dm, a0_v, a1_v).then_inc(rel_sem, 1)
            nc.vector.tensor_max(hm, dm[:, 0::2, :], dm[:, 1::2, :])
            if wm_uses[m] > 0:
                nc.vector.wait_ge(wm_sems[m], 16 * wm_uses[m])
            wp = hm.rearrange("p r (w2 two) -> p r w2 two", two=2)
            nc.vector.tensor_max(wm[m], wp[:, :, :, 0], wp[:, :, :, 1]).then_inc(
                comp_sem, 1
            )
        else:
            # low halves of a0/a1 landed (16 incs each on the slot sems)
            nc.vector.wait_ge(a0_sems[k], 32 * (uses - 1) + 16)
            nc.vector.wait_ge(a1_sems[k], 32 * (uses - 1) + 16)
            if wm_uses[m] > 0:
                nc.vector.wait_ge(wm_sems[m], 16 * wm_uses[m])
            chain(wm[m][:, 0:8, :], 0, 16).then_inc(compl_sem, 1)
            nc.vector.wait_ge(fh_sem, 32)
            chain(wm[m][:, 8:16, :], 16, 16).then_inc(comp_sem, 1)
        wm_uses[m] += 1

    # --- ACT: stores ---------------------------------------------------
    for t in range(NT):
        c, base = tile_slices(t)
        m = t % 2
        if t < NT - 1:
            nc.scalar.wait_ge(comp_sem, t + 1)
            nc.scalar.dma_start(y[c, base // 2 : base // 2 + 32], wm[m]).then_inc(
                wm_sems[m], 16
            )
        else:
            # split final store: low free-rows half as soon as ready.
            yv = y[c, base // 2 : base // 2 + 32].rearrange(
                "dd (q j) ww -> dd q (j ww)", q=4
            )
            nc.scalar.wait_ge(compl_sem, 1)
            nc.scalar.dma_start(yv[:, :, 0:512], wm[m][:, 0:8, :]).then_inc(
                wm_sems[m], 16
            )
            nc.scalar.wait_ge(comp_sem, t + 1)
            nc.scalar.dma_start(yv[:, :, 512:1024], wm[m][:, 8:16, :]).then_inc(
                wm_sems[m], 16
            )
    # wm[1] gets 7 full stores (16 each) + 2 half stores (16 each)
    nc.scalar.wait_ge(wm_sems[0], 16 * (NT // 2))
    nc.scalar.wait_ge(wm_sems[1], 16 * (NT // 2 - 1) + 32)

    nc.compile()
    return nc


def _get_module():
    if "nc" not in _CACHE:
        _CACHE["nc"] = _build_module()
    return _CACHE["nc"]


def _shard_inputs(x: np.ndarray) -> list[dict]:
    B, C, d, h, w = x.shape
    assert (B, C, d, h, w) == (2, 32, 128, 128, 128), x.shape
    xb = np.ascontiguousarray(x, dtype=np.float32).reshape(B * C, d, h, w)
    xb = xb.astype(bfloat16)
    return [{"x": np.ascontiguousarray(xb[i * CPC : (i + 1) * CPC])} for i in range(8)]


def _gather_output(results) -> np.ndarray:
    out = np.concatenate([r["y"] for r in results], axis=0)
    return out.astype(np.float32).reshape(2, 32, D // 2, H // 2, W // 2)


def kernel(x: np.ndarray) -> np.ndarray:
    nc = _get_module()
    in_maps = _shard_inputs(x)
    res = bass_utils.run_bass_kernel_spmd(nc, in_maps, core_ids=list(range(8)))
    return _gather_output(res.results)


# revision 7
# speedup vs baseline: 1.6781x; 1.6781x over previous
"""Raw-bacc MaxPool3d kernel, v8: bf16 + rotating load split + 3-op DVE chain.

bf16 datapath (v7): max-pool commutes with monotone bf16 rounding, so
pooling bf16-rounded inputs yields exactly bf16(true_max) — rel err
<= 2^-8, far inside the 2e-2 gate — and halves HBM traffic, which is the
binding constraint (16 DMA engines/core x ~26.5 GB/s, all saturated).

v8 changes:
- Rotating load split: each parity load is issued as two dma_starts with
  a per-tile rotating plane boundary. The DGE assigns each engine a fixed
  slice of every dma_start, so without rotation engine j always reads the
  same address stripe of every tile; a contended stripe then turns one
  engine into a run-long straggler (observed: +16-19% busy on one engine,
  different engine each run). Rotation spreads any hot stripe over all 16
  engines.
- 3-op DVE chain per tile: D-pair max (a0 vs a1), then H-pair, then
  W-pair straight into the store tile. Same element count as the old
  5-op per-parity chain but one op per reduction level, cutting DVE busy
  ~89 -> ~76 us so DVE never gates the DMA stream.
- Final tile split in half by free rows (loads, DVE chain, and store), so
  the post-last-packet tail is one half-chain + half-store.
"""

import numpy as np
from ml_dtypes import bfloat16

import concourse.bass as bass
from concourse import bacc, mybir
from concourse import bass_utils

CPC = 8
D = H = W = 128
DT = mybir.dt.bfloat16
NSLOT = 4
NT = 16

_CACHE = {}


def _build_module():
    nc = bacc.Bacc("TRN2", target_bir_lowering=False, debug=False, num_devices=8)
    x = nc.dram_tensor("x", [CPC, D, H, W], DT, kind="ExternalInput").ap()
    y = nc.dram_tensor("y", [CPC, D // 2, H // 2, W // 2], DT, kind="ExternalOutput").ap()

    a0 = [nc.alloc_sbuf_tensor(f"a0_{i}", [128, 32, 128], DT).ap() for i in range(NSLOT)]
    a1 = [nc.alloc_sbuf_tensor(f"a1_{i}", [128, 32, 128], DT).ap() for i in range(NSLOT)]
    dm = nc.alloc_sbuf_tensor("dm", [128, 32, 128], DT).ap()
    hm = nc.alloc_sbuf_tensor("hm", [128, 16, 128], DT).ap()
    wm = [nc.alloc_sbuf_tensor(f"wm_{i}", [128, 16, 64], DT).ap() for i in range(2)]

    a0_sems = [nc.alloc_semaphore(f"a0_sem{i}") for i in range(NSLOT)]
    a1_sems = [nc.alloc_semaphore(f"a1_sem{i}") for i in range(NSLOT)]
    wm_sems = [nc.alloc_semaphore(f"wm_sem{i}") for i in range(2)]
    fh_sem = nc.alloc_semaphore("fh_sem")
    rel_sem = nc.alloc_semaphore("rel_sem")
    comp_sem = nc.alloc_semaphore("comp_sem")
    compl_sem = nc.alloc_semaphore("compl_sem")

    def tile_slices(t):
        c, half = divmod(t, 2)
        return c, half * 64

    # Plane-boundary rotation per tile. The DGE deals each dma_start's
    # packets to engines in contiguous blocks of ceil(n_packets/16), so
    # balance requires n_packets % 16 == 0, i.e. plane counts % 4 == 0:
    # rotation offsets must be multiples of 4 (in [4, 28] so both
    # sub-DMAs are nonempty). Different phase per parity for diversity.
    rot0 = [4 * (1 + (3 * t) % 7) for t in range(NT)]
    rot1 = [4 * (1 + (3 * t + 4) % 7) for t in range(NT)]

    # --- SP: loads -----------------------------------------------------
    for t in range(NT):
        c, base = tile_slices(t)
        k = t % NSLOT
        if t >= NSLOT:
            nc.sync.wait_ge(rel_sem, t - NSLOT + 1)
        if t < NT - 1:
            for par, buf, sems, rr in (
                (0, a0, a0_sems, rot0[t]),
                (1, a1, a1_sems, rot1[t]),
            ):
                lo = base + par
                nc.sync.dma_start(
                    buf[k][4 * rr : 128], x[c, lo + 2 * rr : base + 64 : 2]
                ).then_inc(sems[k], 16)
                nc.sync.dma_start(
                    buf[k][0 : 4 * rr], x[c, lo : lo + 2 * rr : 2]
                ).then_inc(sems[k], 16)
        else:
            # final tile: halve every load by free rows (h%32 stripes) so
            # the DVE chain and store can run on the first half while the
            # second is in flight.
            for par, buf, sems in ((0, a0, a0_sems), (1, a1, a1_sems)):
                src = x[c, base + par : base + 64 : 2]
                sr = src.rearrange("d (hb r) w -> d hb (r w)", hb=4)
                nc.sync.dma_start(buf[k][:, 0:16, :], sr[:, :, 0:2048]).then_inc(
                    sems[k], 16
                )
                nc.sync.dma_start(buf[k][:, 16:32, :], sr[:, :, 2048:4096]).then_inc(
                    fh_sem, 16
                )

    # --- DVE: D-pair, H-pair, W-pair max -------------------------------
    def chain(dst, rows, n):
        # rows: dm/a row offset; n: dm/a row count (out rows n//4 in dst)
        dmv = dm[:, rows : rows + n, :]
        nc.vector.tensor_max(dmv, a0_v[:, rows : rows + n, :], a1_v[:, rows : rows + n, :])
        hv = hm[:, rows // 2 : rows // 2 + n // 2, :]
        nc.vector.tensor_max(hv, dmv[:, 0::2, :], dmv[:, 1::2, :])
        wp = hv.rearrange("p r (w2 two) -> p r w2 two", two=2)
        return nc.vector.tensor_max(dst, wp[:, :, :, 0], wp[:, :, :, 1])

    wm_uses = [0, 0]
    for t in range(NT):
        k = t % NSLOT
        m = t % 2
        uses = t // NSLOT + 1
        a0_v, a1_v = a0[k], a1[k]
        if t < NT - 1:
            nc.vector.wait_ge(a0_sems[k], 32 * uses)
            nc.vector.wait_ge(a1_sems[k], 32 * uses)
            nc.vector.tensor_max(dm, a0_v, a1_v).then_inc(rel_sem, 1)
            nc.vector.tensor_max(hm, dm[:, 0::2, :], dm[:, 1::2, :])
            if wm_uses[m] > 0:
                nc.vector.wait_ge(wm_sems[m], 16 * wm_uses[m])
            wp = hm.rearrange("p r (w2 two) -> p r w2 two", two=2)
            nc.vector.tensor_max(wm[m], wp[:, :, :, 0], wp[:, :, :, 1]).then_inc(
                comp_sem, 1
            )
        else:
            # low halves of a0/a1 landed (16 incs each on the slot sems)
            nc.vector.wait_ge(a0_sems[k], 32 * (uses - 1) + 16)
            nc.vector.wait_ge(a1_sems[k], 32 * (uses - 1) + 16)
            if wm_uses[m] > 0:
                nc.vector.wait_ge(wm_sems[m], 16 * wm_uses[m])
            chain(wm[m][:, 0:8, :], 0, 16).then_inc(compl_sem, 1)
            nc.vector.wait_ge(fh_sem, 32)
            chain(wm[m][:, 8:16, :], 16, 16).then_inc(comp_sem, 1)
        wm_uses[m] += 1

    # --- ACT: stores ---------------------------------------------------
    for t in range(NT):
        c, base = tile_slices(t)
        m = t % 2
        if t < NT - 1:
            nc.scalar.wait_ge(comp_sem, t + 1)
            nc.scalar.dma_start(y[c, base // 2 : base // 2 + 32], wm[m]).then_inc(
                wm_sems[m], 16
            )
        else:
            # split final store: low free-rows half as soon as ready.
            yv = y[c, base // 2 : base // 2 + 32].rearrange(
                "dd (q j) ww -> dd q (j ww)", q=4
            )
            nc.scalar.wait_ge(compl_sem, 1)
            nc.scalar.dma_start(yv[:, :, 0:512], wm[m][:, 0:8, :]).then_inc(
                wm_sems[m], 16
            )
            nc.scalar.wait_ge(comp_sem, t + 1)
            nc.scalar.dma_start(yv[:, :, 512:1024], wm[m][:, 8:16, :]).then_inc(
                wm_sems[m], 16
            )
    # wm[1] gets 7 full stores (16 each) + 2 half stores (16 each)
    nc.scalar.wait_ge(wm_sems[0], 16 * (NT // 2))
    nc.scalar.wait_ge(wm_sems[1], 16 * (NT // 2 - 1) + 32)

    nc.compile()
    return nc


def _get_module():
    if "nc" not in _CACHE:
        _CACHE["nc"] = _build_module()
    return _CACHE["nc"]


def _shard_inputs(x: np.ndarray) -> list[dict]:
    B, C, d, h, w = x.shape
    assert (B, C, d, h, w) == (2, 32, 128, 128, 128), x.shape
    xb = np.ascontiguousarray(x, dtype=np.float32).reshape(B * C, d, h, w)
    xb = xb.astype(bfloat16)
    return [{"x": np.ascontiguousarray(xb[i * CPC : (i + 1) * CPC])} for i in range(8)]


def _gather_output(results) -> np.ndarray:
    out = np.concatenate([r["y"] for r in results], axis=0)
    return out.astype(np.float32).reshape(2, 32, D // 2, H // 2, W // 2)


def kernel(x: np.ndarray) -> np.ndarray:
    nc = _get_module()
    in_maps = _shard_inputs(x)
    res = bass_utils.run_bass_kernel_spmd(nc, in_maps, core_ids=list(range(8)))
    return _gather_output(res.results)


# revision 10
# speedup vs baseline: 2.6284x; 1.5662x over previous
"""Raw-bacc MaxPool3d kernel, v8: bf16 + rotating load split + 3-op DVE chain.

bf16 datapath (v7): max-pool commutes with monotone bf16 rounding, so
pooling bf16-rounded inputs yields exactly bf16(true_max) — rel err
<= 2^-8, far inside the 2e-2 gate — and halves HBM traffic, which is the
binding constraint (16 DMA engines/core x ~26.5 GB/s, all saturated).

v8 changes:
- Rotating load split: each parity load is issued as two dma_starts with
  a per-tile rotating plane boundary. The DGE assigns each engine a fixed
  slice of every dma_start, so without rotation engine j always reads the
  same address stripe of every tile; a contended stripe then turns one
  engine into a run-long straggler (observed: +16-19% busy on one engine,
  different engine each run). Rotation spreads any hot stripe over all 16
  engines.
- 3-op DVE chain per tile: D-pair max (a0 vs a1), then H-pair, then
  W-pair straight into the store tile. Same element count as the old
  5-op per-parity chain but one op per reduction level, cutting DVE busy
  ~89 -> ~76 us so DVE never gates the DMA stream.
- Final tile split in half by free rows (loads, DVE chain, and store), so
  the post-last-packet tail is one half-chain + half-store.
"""

import numpy as np
from ml_dtypes import bfloat16

import concourse.bass as bass
from concourse import bacc, mybir
from concourse import bass_utils

CPC = 8
D = H = W = 128
DT = mybir.dt.bfloat16
NSLOT = 4
NT = 16

_CACHE = {}


def _build_module():
    nc = bacc.Bacc("TRN2", target_bir_lowering=False, debug=False, num_devices=8)
    x = nc.dram_tensor("x", [CPC, D, H, W], DT, kind="ExternalInput").ap()
    y = nc.dram_tensor("y", [CPC, D // 2, H // 2, W // 2], DT, kind="ExternalOutput").ap()

    a0 = [nc.alloc_sbuf_tensor(f"a0_{i}", [128, 32, 128], DT).ap() for i in range(NSLOT)]
    a1 = [nc.alloc_sbuf_tensor(f"a1_{i}", [128, 32, 128], DT).ap() for i in range(NSLOT)]
    dm = nc.alloc_sbuf_tensor("dm", [128, 32, 128], DT).ap()
    hm = nc.alloc_sbuf_tensor("hm", [128, 16, 128], DT).ap()
    wm = [nc.alloc_sbuf_tensor(f"wm_{i}", [128, 16, 64], DT).ap() for i in range(2)]

    a0_sems = [nc.alloc_semaphore(f"a0_sem{i}") for i in range(NSLOT)]
    a1_sems = [nc.alloc_semaphore(f"a1_sem{i}") for i in range(NSLOT)]
    wm_sems = [nc.alloc_semaphore(f"wm_sem{i}") for i in range(2)]
    fh_sem = nc.alloc_semaphore("fh_sem")
    rel_sem = nc.alloc_semaphore("rel_sem")
    comp_sem = nc.alloc_semaphore("comp_sem")
    compl_sem = nc.alloc_semaphore("compl_sem")

    def tile_slices(t):
        c, half = divmod(t, 2)
        return c, half * 64

    # --- SP: loads -----------------------------------------------------
    # Uniform full-tile dma_starts only: the DGE deals a dma_start's
    # packets across the 16 engines evenly ONLY for this uniform shape;
    # split/partition-offset dsts measured 2-3x slower (unbalanced deal +
    # reduced per-packet rate).
    for t in range(NT):
        c, base = tile_slices(t)
        k = t % NSLOT
        if t >= NSLOT:
            nc.sync.wait_ge(rel_sem, t - NSLOT + 1)
        if t < NT - 1:
            nc.sync.dma_start(a0[k], x[c, base : base + 64 : 2]).then_inc(
                a0_sems[k], 16
            )
            nc.sync.dma_start(a1[k], x[c, base + 1 : base + 64 : 2]).then_inc(
                a1_sems[k], 16
            )
        else:
            # final tile: halve every load by free rows (h%32 stripes) so
            # the DVE chain and store can run on the first half while the
            # second is in flight.
            for par, buf, sems in ((0, a0, a0_sems), (1, a1, a1_sems)):
                src = x[c, base + par : base + 64 : 2]
                sr = src.rearrange("d (hb r) w -> d hb (r w)", hb=4)
                nc.sync.dma_start(buf[k][:, 0:16, :], sr[:, :, 0:2048]).then_inc(
                    sems[k], 16
                )
                nc.sync.dma_start(buf[k][:, 16:32, :], sr[:, :, 2048:4096]).then_inc(
                    fh_sem, 16
                )

    # --- DVE: D-pair, H-pair, W-pair max -------------------------------
    def chain(dst, rows, n):
        # rows: dm/a row offset; n: dm/a row count (out rows n//4 in dst)
        dmv = dm[:, rows : rows + n, :]
        nc.vector.tensor_max(dmv, a0_v[:, rows : rows + n, :], a1_v[:, rows : rows + n, :])
        hv = hm[:, rows // 2 : rows // 2 + n // 2, :]
        nc.vector.tensor_max(hv, dmv[:, 0::2, :], dmv[:, 1::2, :])
        wp = hv.rearrange("p r (w2 two) -> p r w2 two", two=2)
        return nc.vector.tensor_max(dst, wp[:, :, :, 0], wp[:, :, :, 1])

    wm_uses = [0, 0]
    for t in range(NT):
        k = t % NSLOT
        m = t % 2
        uses = t // NSLOT + 1
        a0_v, a1_v = a0[k], a1[k]
        if t < NT - 1:
            nc.vector.wait_ge(a0_sems[k], 16 * uses)
            nc.vector.wait_ge(a1_sems[k], 16 * uses)
            nc.vector.tensor_max(dm, a0_v, a1_v).then_inc(rel_sem, 1)
            nc.vector.tensor_max(hm, dm[:, 0::2, :], dm[:, 1::2, :])
            if wm_uses[m] > 0:
                nc.vector.wait_ge(wm_sems[m], 16 * wm_uses[m])
            wp = hm.rearrange("p r (w2 two) -> p r w2 two", two=2)
            nc.vector.tensor_max(wm[m], wp[:, :, :, 0], wp[:, :, :, 1]).then_inc(
                comp_sem, 1
            )
        else:
            # low halves of a0/a1 landed (16 incs each on the slot sems)
            nc.vector.wait_ge(a0_sems[k], 16 * (uses - 1) + 16)
            nc.vector.wait_ge(a1_sems[k], 16 * (uses - 1) + 16)
            if wm_uses[m] > 0:
                nc.vector.wait_ge(wm_sems[m], 16 * wm_uses[m])
            chain(wm[m][:, 0:8, :], 0, 16).then_inc(compl_sem, 1)
            nc.vector.wait_ge(fh_sem, 32)
            chain(wm[m][:, 8:16, :], 16, 16).then_inc(comp_sem, 1)
        wm_uses[m] += 1

    # --- ACT: stores ---------------------------------------------------
    for t in range(NT):
        c, base = tile_slices(t)
        m = t % 2
        if t < NT - 1:
            nc.scalar.wait_ge(comp_sem, t + 1)
            nc.scalar.dma_start(y[c, base // 2 : base // 2 + 32], wm[m]).then_inc(
                wm_sems[m], 16
            )
        else:
            # split final store: low free-rows half as soon as ready.
            yv = y[c, base // 2 : base // 2 + 32].rearrange(
                "dd (q j) ww -> dd q (j ww)", q=4
            )
            nc.scalar.wait_ge(compl_sem, 1)
            nc.scalar.dma_start(yv[:, :, 0:512], wm[m][:, 0:8, :]).then_inc(
                wm_sems[m], 16
            )
            nc.scalar.wait_ge(comp_sem, t + 1)
            nc.scalar.dma_start(yv[:, :, 512:1024], wm[m][:, 8:16, :]).then_inc(
                wm_sems[m], 16
            )
    # wm[1] gets 7 full stores (16 each) + 2 half stores (16 each)
    nc.scalar.wait_ge(wm_sems[0], 16 * (NT // 2))
    nc.scalar.wait_ge(wm_sems[1], 16 * (NT // 2 - 1) + 32)

    nc.compile()
    return nc


def _get_module():
    if "nc" not in _CACHE:
        _CACHE["nc"] = _build_module()
    return _CACHE["nc"]


def _shard_inputs(x: np.ndarray) -> list[dict]:
    B, C, d, h, w = x.shape
    assert (B, C, d, h, w) == (2, 32, 128, 128, 128), x.shape
    xb = np.ascontiguousarray(x, dtype=np.float32).reshape(B * C, d, h, w)
    xb = xb.astype(bfloat16)
    return [{"x": np.ascontiguousarray(xb[i * CPC : (i + 1) * CPC])} for i in range(8)]


def _gather_output(results) -> np.ndarray:
    out = np.concatenate([r["y"] for r in results], axis=0)
    return out.astype(np.float32).reshape(2, 32, D // 2, H // 2, W // 2)


def kernel(x: np.ndarray) -> np.ndarray:
    nc = _get_module()
    in_maps = _shard_inputs(x)
    res = bass_utils.run_bass_kernel_spmd(nc, in_maps, core_ids=list(range(8)))
    return _gather_output(res.results)
